# revision 7
# baseline (speedup 1.0000x reference)
"""Two-layer GAT (PyG GATConv semantics) on 8 Trainium2 NeuronCores.

v3 design:
- Edge/dst sharding: host sorts edges by dst; core c owns dst nodes
  [c*SHARD,(c+1)*SHARD) and their incoming edges, grouped per
  128-dst-node tile into KT subtiles of 128 edge slots.
- Batched gathers via InstDMAGatherAnt (nc.gpsimd.dma_gather): one SWDGE
  instruction per (tile, half-table group) instead of one per subtile.
  dma_gather needs int16 indices, so the node table is split at
  HALF=NP/2 and each tile's edges are regrouped: slots [0,KA) gather
  from rows <HALF, slots [KA,KT) from rows >=HALF (index biased).
  Index layout: k-major flat order wrapped into 16 partitions and
  replicated 8x down partitions (one copy per GpSimd q7 core).
- Table rows are 256B-multiple (gather constraint):
  layer1 [NP,256] bf16: [h0(64)|1|h1(64)|1|as 2xf32(4)|ad 2xbf16(2)|pad]
  layer2 [NP,128] bf16: [h2(64)|1|pad|as2 f32(2)|ad2 bf16(1)|pad]
  The literal 1.0 columns make [h|1] contiguous so one matmul computes
  segment numerator and denominator together.
- ad[dst] expansion: host ships transposed one-hots st2 (bf16 0/1,
  [NT,P,KT*P]); per subtile one tiny PE matmul st2_k.T @ adn. ad is
  stored bf16: softmax is shift-invariant per segment so ad rounding
  only acts through the leaky-relu kink.
- z = as[src]+ad[dst] fp32 via bitcast strided views, one DVE op/tile;
  Lrelu+Exp whole-tile on the scalar engine.
- Scaled one-hot fused in one DVE op per (subtile, head):
  st = (iota == dst_rel) * ex; acc matmul bf16 (1cy/row).
- Phase0 row-major via matmul(lhsT=xT_tile, rhs=W), no transposes.
- AllGathers bf16 into Shared scratchpad.
"""

import sys

for _p in ("/opt/trn_rl_repo",):
    if _p not in sys.path:
        sys.path.insert(0, _p)

import numpy as np
import ml_dtypes

P = 128
NEG_SLOPE = 0.2
EPS = 1e-16
R1 = 256          # layer-1 gather row elems (bf16)
C1 = 136          # layer-1 compact row elems actually written
R2 = 128          # layer-2 gather row elems
C2 = 70           # layer-2 compact row elems (even pitch)
G = 64
NQ = 4            # SWDGE queues


def _to_bf(a):
    return np.asarray(a, np.float32).astype(ml_dtypes.bfloat16)


def _wrap16(mat):
    """[T, n] k-major flat idx -> [T, 128, n/16] (16-wrap, 8x replicated)."""
    T, n = mat.shape
    blk = mat.reshape(T, n // 16, 16).transpose(0, 2, 1).astype(np.int16)
    return np.ascontiguousarray(np.tile(blk, (1, 8, 1)))


def host_prep(inputs, cores=8):
    x = np.asarray(inputs["x"], dtype=np.float32)
    edge_index = np.asarray(inputs["edge_index"])
    batch = np.asarray(inputs["batch"])
    W1 = np.asarray(inputs["W1"], dtype=np.float32)
    a_src1 = np.asarray(inputs["a_src1"], dtype=np.float32)
    a_dst1 = np.asarray(inputs["a_dst1"], dtype=np.float32)
    b1 = np.asarray(inputs["b1"], dtype=np.float32)
    W2 = np.asarray(inputs["W2"], dtype=np.float32)
    a_src2 = np.asarray(inputs["a_src2"], dtype=np.float32)
    a_dst2 = np.asarray(inputs["a_dst2"], dtype=np.float32)
    b2 = np.asarray(inputs["b2"], dtype=np.float32)
    Wc = np.asarray(inputs["Wc"], dtype=np.float32)
    bc = np.asarray(inputs["bc"], dtype=np.float32)

    N, F = x.shape
    H1 = a_src1.shape[0]            # 2
    HID = a_src1.shape[1]           # 64
    D1 = H1 * HID                   # 128
    NCLS = Wc.shape[1]

    NP = ((N + cores * P - 1) // (cores * P)) * (cores * P)
    SHARD = NP // cores
    NT = SHARD // P

    # --- weights ---
    As1 = np.zeros((D1, H1), np.float32)
    Ad1 = np.zeros((D1, H1), np.float32)
    for h in range(H1):
        As1[h * HID:(h + 1) * HID, h] = a_src1[h]
        Ad1[h * HID:(h + 1) * HID, h] = a_dst1[h]
    Wh = _to_bf(W1)
    Wsd = _to_bf(np.concatenate([W1 @ As1, W1 @ Ad1], axis=1))  # [F,4]
    W2aug = _to_bf(np.concatenate([W2, W2 @ a_src2.T, W2 @ a_dst2.T], axis=1))

    # --- edges sorted by dst, split by src half, grouped per dst tile ---
    src = edge_index[0].astype(np.int64)
    dst = edge_index[1].astype(np.int64)
    order = np.argsort(dst, kind="stable")
    ss = src[order]
    ds = dst[order]
    rel = (ds % P).astype(np.float32)
    tile_of = (ds // P).astype(np.int64)
    n_tiles = NP // P
    counts = np.bincount(tile_of, minlength=n_tiles)
    starts = np.cumsum(counts) - counts

    # N-way split of the node table: dma_gather caps num_idxs at 1024
    # and indices are int16, so each group must satisfy rows<=32767 and
    # max edges/tile/group <= 1024.
    NS = 3
    while True:
        S = -(-NP // NS)
        grp = np.minimum(ss // S, NS - 1)
        nQ = np.zeros((n_tiles, NS), np.int64)
        for t in range(n_tiles):
            nQ[t] = np.bincount(grp[starts[t]:starts[t] + counts[t]],
                                minlength=NS)
        Ks = [int(np.ceil(nQ[:, q].max() / P)) for q in range(NS)]
        if S <= 32767 and all(k <= 8 for k in Ks):
            break
        NS += 1
    KT = sum(Ks)
    koff = np.cumsum([0] + Ks)

    idx_all = [np.zeros((n_tiles, Ks[q] * P), np.int16) for q in range(NS)]
    relfull = np.full((n_tiles, P, KT), 255.0, np.float32)
    for t in range(n_tiles):
        e0 = starts[t]
        e1 = e0 + counts[t]
        s_t = ss[e0:e1]
        r_t = rel[e0:e1]
        g_t = grp[e0:e1]
        for q in range(NS):
            m = g_t == q
            sq = s_t[m] - q * S
            rq = r_t[m]
            idx_all[q][t, :len(sq)] = sq.astype(np.int16)
            iq = np.arange(len(sq))
            relfull[t, iq % P, koff[q] + iq // P] = rq

    # transposed one-hots st2[t][d, k*128+e] = (relfull[t,e,k]==d), bf16
    st2_bool = (np.arange(P, dtype=np.float32)[None, :, None, None]
                == relfull.transpose(0, 2, 1)[:, None, :, :])
    st2_all = st2_bool.astype(ml_dtypes.bfloat16).reshape(n_tiles, P, KT * P)
    del st2_bool

    idx_w = [_wrap16(idx_all[q]) for q in range(NS)]

    bpad = np.full(NP, 255.0, np.float32)
    bpad[:N] = batch.astype(np.float32)
    xpad = np.zeros((NP, F), np.float32)
    xpad[:N] = x

    iota128 = np.tile(np.arange(P, dtype=np.float32), (P, 1))
    iota64 = np.tile(np.arange(G, dtype=np.float32), (P, 1))
    ident_bf = np.eye(P, dtype=np.float32).astype(ml_dtypes.bfloat16)
    ident_f = np.eye(P, dtype=np.float32)
    b1rep = np.tile(b1, (P, 1))
    b2rep = np.tile(b2, (P, 1))
    bcrep = np.tile(bc, (G, 1))
    ones_col = np.ones((P, 1), np.float32)

    shared = {
        "Wh": Wh, "Wsd": Wsd, "W2aug": W2aug, "iota128": iota128,
        "iota64": iota64, "ident_bf": ident_bf, "ident_f": ident_f,
        "b1rep": b1rep, "b2rep": b2rep, "Wc": Wc, "bcrep": bcrep,
        "ones_col": ones_col,
    }
    per_core = []
    for c in range(cores):
        lo = c * SHARD
        t0, t1 = c * NT, (c + 1) * NT
        dr_all = relfull[t0:t1].transpose(1, 0, 2).reshape(P, NT * KT)
        pc = {
            **shared,
            "xT": np.ascontiguousarray(_to_bf(xpad[lo:lo + SHARD].T)),
            "dr_all": np.ascontiguousarray(dr_all),
            "st2": np.ascontiguousarray(st2_all[t0:t1]),
            "batchv": np.ascontiguousarray(
                bpad[lo:lo + SHARD].reshape(NT, P, 1)),
        }
        for q in range(NS):
            iqw = idx_w[q][t0:t1].transpose(1, 0, 2).reshape(
                P, NT * Ks[q] * 8)
            pc[f"idx{q}"] = np.ascontiguousarray(iqw)
        per_core.append(pc)

    cfg = dict(N=N, F=F, H1=H1, HID=HID, D1=D1, NCLS=NCLS, G=G, NP=NP,
               SHARD=SHARD, NT=NT, NS=NS, S=S, Ks=Ks, KT=KT,
               cores=cores)
    return cfg, per_core


def build_program(cfg):
    import concourse.bacc as bacc
    import concourse.bass as bass
    import concourse.mybir as mybir
    import concourse.tile as tile

    f32 = mybir.dt.float32
    bf16 = mybir.dt.bfloat16
    i16 = mybir.dt.int16
    AF = mybir.ActivationFunctionType
    OP = mybir.AluOpType

    F, H1, HID, D1 = cfg["F"], cfg["H1"], cfg["HID"], cfg["D1"]
    NCLS = cfg["NCLS"]
    NP, SHARD, NT = cfg["NP"], cfg["SHARD"], cfg["NT"]
    NS, S, Ks, KT = cfg["NS"], cfg["S"], cfg["Ks"], cfg["KT"]
    koff = [0]
    for k in Ks:
        koff.append(koff[-1] + k)
    cores = cfg["cores"]

    nc = bacc.Bacc("TRN2", target_bir_lowering=False, debug=False,
                   num_swdge_queues=NQ)

    xT = nc.dram_tensor("xT", [F, SHARD], bf16, kind="ExternalInput")
    idx_d = [nc.dram_tensor(f"idx{q}", [P, NT * Ks[q] * 8], i16,
                            kind="ExternalInput") for q in range(NS)]
    dr_all_d = nc.dram_tensor("dr_all", [P, NT * KT], f32,
                              kind="ExternalInput")
    st2_d = nc.dram_tensor("st2", [NT, P, KT * P], bf16,
                           kind="ExternalInput")
    batchv = nc.dram_tensor("batchv", [NT, P, 1], f32, kind="ExternalInput")
    Wh_d = nc.dram_tensor("Wh", [F, D1], bf16, kind="ExternalInput")
    Wsd_d = nc.dram_tensor("Wsd", [F, 4], bf16, kind="ExternalInput")
    W2aug = nc.dram_tensor("W2aug", [D1, HID + 2], bf16, kind="ExternalInput")
    iota128 = nc.dram_tensor("iota128", [P, P], f32, kind="ExternalInput")
    iota64 = nc.dram_tensor("iota64", [P, G], f32, kind="ExternalInput")
    ident_bf = nc.dram_tensor("ident_bf", [P, P], bf16, kind="ExternalInput")
    ident_f = nc.dram_tensor("ident_f", [P, P], f32, kind="ExternalInput")
    b1rep = nc.dram_tensor("b1rep", [P, D1], f32, kind="ExternalInput")
    b2rep = nc.dram_tensor("b2rep", [P, HID], f32, kind="ExternalInput")
    Wc = nc.dram_tensor("Wc", [HID, NCLS], f32, kind="ExternalInput")
    bcrep = nc.dram_tensor("bcrep", [G, NCLS], f32, kind="ExternalInput")
    ones_col = nc.dram_tensor("ones_col", [P, 1], f32, kind="ExternalInput")

    y = nc.dram_tensor("y", [G, NCLS], f32, kind="ExternalOutput")

    def view3(t, off, blk, n, w):
        b = t[:]
        return bass.AP(b.tensor, b.offset + off,
                       [[b.ap[0][0], P], [blk, n], [1, w]])

    with tile.TileContext(nc) as tc:
        with (
            tc.tile_pool(name="const", bufs=1) as cpool,
            tc.tile_pool(name="gat", bufs=3) as gpool,
            tc.tile_pool(name="st2p", bufs=3) as st2pool,
            tc.tile_pool(name="work", bufs=3) as wpool,
            tc.tile_pool(name="small", bufs=3) as spool,
            tc.tile_pool(name="pbig", bufs=2, space="PSUM") as pbig,
            tc.tile_pool(name="pacc", bufs=2, space="PSUM") as pacc,
            tc.tile_pool(name="padx", bufs=2, space="PSUM") as padx,
            tc.tile_pool(name="pcls", bufs=1, space="PSUM") as pcls,
            tc.tile_pool(name="ppool", bufs=1, space="PSUM") as ppool,
            tc.tile_pool(name="dram", bufs=1, space="DRAM") as dpool,
        ):
            def cload(ap, shape, dt, tag):
                t = cpool.tile(shape, dt, tag=tag)
                nc.sync.dma_start(out=t[:], in_=ap[:])
                return t

            wh_sb = cload(Wh_d, [F, D1], bf16, "wh")
            wsd_sb = cload(Wsd_d, [F, 4], bf16, "wsd")
            w2_sb = cload(W2aug, [D1, HID + 2], bf16, "w2")
            io128_sb = cload(iota128, [P, P], f32, "io128")
            io64_sb = cload(iota64, [P, G], f32, "io64")
            idb_sb = cload(ident_bf, [P, P], bf16, "idb")
            idf_sb = cload(ident_f, [P, P], f32, "idf")
            b1_sb = cload(b1rep, [P, D1], f32, "b1")
            b2_sb = cload(b2rep, [P, HID], f32, "b2")
            wc_sb = cload(Wc, [HID, NCLS], f32, "wc")
            bc_sb = cload(bcrep, [G, NCLS], f32, "bc")
            ones_sb = cload(ones_col, [P, 1], f32, "ones")
            idx_sb = [cload(idx_d[q], [P, NT * Ks[q] * 8], i16, f"i{q}")
                      for q in range(NS)]
            dr_sb = cload(dr_all_d, [P, NT * KT], f32, "drall")

            h1_shard = dpool.tile([SHARD, R1], bf16, tag="h1s")
            h1_full = dpool.tile([NP, R1], bf16, tag="h1f",
                                 addr_space="Shared")
            h2_shard = dpool.tile([SHARD, R2], bf16, tag="h2s")
            h2_full = dpool.tile([NP, R2], bf16, tag="h2f",
                                 addr_space="Shared")
            pool_in = dpool.tile([G, HID + 1], f32, tag="pin")
            pool_out = dpool.tile([G, HID + 1], f32, tag="pout")

            groups = [list(range(cores))]

            # ===== phase 0: layer-1 table rows = x @ [W1 | W1As | W1Ad] ====
            for t in range(NT):
                xt = wpool.tile([F, P], bf16, tag="xt")
                nc.sync.dma_start(out=xt[:], in_=xT[:, t * P:(t + 1) * P])
                ph = pbig.tile([P, 194], f32, tag="big")
                nc.tensor.matmul(out=ph[:, 0:D1], lhsT=xt[:], rhs=wh_sb[:],
                                 start=True, stop=True)
                nc.tensor.matmul(out=ph[:, D1:D1 + 4], lhsT=xt[:],
                                 rhs=wsd_sb[:], start=True, stop=True)
                row = wpool.tile([P, C1], bf16, tag="row1")
                nc.vector.tensor_copy(out=row[:, 0:64], in_=ph[:, 0:64])
                nc.vector.tensor_copy(out=row[:, 65:129], in_=ph[:, 64:128])
                nc.vector.memset(row[:, 64:65], 1.0)
                nc.vector.memset(row[:, 129:130], 1.0)
                nc.vector.tensor_copy(out=row[:, 130:134].bitcast(f32),
                                      in_=ph[:, D1:D1 + 2])
                nc.vector.tensor_copy(out=row[:, 134:136],
                                      in_=ph[:, D1 + 2:D1 + 4])
                nc.sync.dma_start(out=h1_shard[t * P:(t + 1) * P, 0:C1],
                                  in_=row[:])

            nc.gpsimd.collective_compute(
                "AllGather", mybir.AluOpType.bypass,
                replica_groups=groups,
                ins=[h1_shard.opt()], outs=[h1_full.opt()])

            # =================== edge phase (both layers) ==================
            def edge_layer(table_full, shard, R, heads, as_off, ad_off, post):
                tabs = [table_full[q * S:min((q + 1) * S, NP), :]
                        for q in range(NS)]
                for t in range(NT):
                    g = gpool.tile([P, KT * R], bf16, tag=f"g{R}")
                    for q in range(NS):
                        nc.gpsimd.dma_gather(
                            out_ap=view3(g, koff[q] * R, R, Ks[q], R),
                            in_ap=tabs[q],
                            idxs_ap=idx_sb[q][:, t * Ks[q] * 8:
                                              (t + 1) * Ks[q] * 8],
                            num_idxs=Ks[q] * P, num_idxs_reg=Ks[q] * P,
                            elem_size=R, queue_num=(NS * t + q) % NQ)
                    st2_sb = st2pool.tile([P, KT * P], bf16, tag="st2")
                    nc.sync.dma_start(out=st2_sb[:], in_=st2_d[t])
                    adn = spool.tile([P, heads], bf16, tag=f"adn{R}")
                    nc.sync.dma_start(
                        out=adn[:],
                        in_=shard[t * P:(t + 1) * P, ad_off:ad_off + heads])
                    adx = padx.tile([P, KT * H1], f32, tag="adx")
                    for k in range(KT):
                        nc.tensor.matmul(
                            out=adx[:, k * heads:(k + 1) * heads],
                            lhsT=st2_sb[:, k * P:(k + 1) * P], rhs=adn[:],
                            start=True, stop=True)
                    z = spool.tile([P, KT * heads], f32, tag=f"z{R}")
                    nc.vector.tensor_tensor(
                        out=view3(z, 0, heads, KT, heads),
                        in0=view3(g, as_off, R, KT, 2 * heads).bitcast(f32),
                        in1=view3(adx, 0, heads, KT, heads),
                        op=OP.add)
                    zl = spool.tile([P, KT * heads], f32, tag=f"zl{R}")
                    nc.vector.tensor_scalar_mul(out=zl[:], in0=z[:],
                                                scalar1=NEG_SLOPE)
                    zm = spool.tile([P, KT * heads], f32, tag=f"zm{R}")
                    nc.vector.tensor_tensor(out=zm[:], in0=z[:], in1=zl[:],
                                            op=OP.max)
                    ex = spool.tile([P, KT * heads], f32, tag=f"ex{R}")
                    nc.scalar.activation(out=ex[:], in_=zm[:], func=AF.Exp)
                    acc = pacc.tile([P, H1 * 65], f32, tag="acc")
                    for k in range(KT):
                        for h in range(heads):
                            sth = wpool.tile([P, P], bf16, tag="sth")
                            nc.vector.tensor_scalar(
                                out=sth[:], in0=io128_sb[:],
                                scalar1=dr_sb[:, t * KT + k:t * KT + k + 1],
                                scalar2=ex[:, k * heads + h:k * heads + h + 1],
                                op0=OP.is_equal, op1=OP.mult)
                            nc.tensor.matmul(
                                out=acc[:, h * 65:(h + 1) * 65], lhsT=sth[:],
                                rhs=g[:, k * R + h * 65:k * R + h * 65 + 65],
                                start=(k == 0), stop=(k == KT - 1))
                    post(t, acc)

            # ---- layer-1 post: divide, +b1, ELU, project to layer-2 row ---
            def post1(t, acc):
                den = spool.tile([P, H1], f32, tag="den")
                nc.vector.tensor_scalar_add(
                    out=den[:], in0=view3(acc, 64, 65, H1, 1), scalar1=EPS)
                rec = spool.tile([P, H1], f32, tag="rec")
                nc.vector.reciprocal(out=rec[:], in_=den[:])
                o = wpool.tile([P, D1], f32, tag="o")
                for h in range(H1):
                    nc.vector.tensor_scalar_mul(
                        out=o[:, h * 64:(h + 1) * 64],
                        in0=acc[:, h * 65:h * 65 + 64],
                        scalar1=rec[:, h:h + 1])
                nc.vector.tensor_tensor(out=o[:], in0=o[:], in1=b1_sb[:],
                                        op=OP.add)
                m0 = wpool.tile([P, D1], f32, tag="m0")
                nc.vector.tensor_scalar_min(out=m0[:], in0=o[:], scalar1=0.0)
                em = wpool.tile([P, D1], f32, tag="em")
                nc.scalar.activation(out=em[:], in_=m0[:], func=AF.Exp)
                nc.vector.tensor_scalar_add(out=em[:], in0=em[:], scalar1=-1.0)
                hb = wpool.tile([P, D1], f32, tag="hb")
                nc.vector.tensor_tensor(out=hb[:], in0=o[:], in1=em[:],
                                        op=OP.max)
                hT = pbig.tile([P, 194], f32, tag="big")
                nc.tensor.transpose(out=hT[:, 0:P], in_=hb[:],
                                    identity=idf_sb[:])
                hTs = wpool.tile([P, P], bf16, tag="hTs")
                nc.vector.tensor_copy(out=hTs[:], in_=hT[:, 0:P])
                nc.tensor.matmul(out=hT[:, 128:128 + HID + 2], lhsT=hTs[:],
                                 rhs=w2_sb[:], start=True, stop=True)
                row2 = wpool.tile([P, C2], bf16, tag="row2")
                nc.vector.tensor_copy(out=row2[:, 0:64],
                                      in_=hT[:, 128:128 + 64])
                nc.vector.memset(row2[:, 64:65], 1.0)
                nc.vector.memset(row2[:, 65:66], 0.0)
                nc.vector.memset(row2[:, 69:70], 0.0)
                nc.vector.tensor_copy(
                    out=row2[:, 66:68].bitcast(f32),
                    in_=hT[:, 128 + HID:128 + HID + 1])
                nc.vector.tensor_copy(
                    out=row2[:, 68:69],
                    in_=hT[:, 128 + HID + 1:128 + HID + 2])
                nc.sync.dma_start(out=h2_shard[t * P:(t + 1) * P, 0:C2],
                                  in_=row2[:])

            edge_layer(h1_full, h1_shard, R1, H1, 130, 134, post1)

            nc.gpsimd.collective_compute(
                "AllGather", mybir.AluOpType.bypass,
                replica_groups=groups,
                ins=[h2_shard.opt()], outs=[h2_full.opt()])

            # ---- layer-2 post: divide, +b2, pool accumulate ----
            pool_ps = ppool.tile([G, HID + 1], f32, tag="pool_ps")

            def post2(t, acc):
                den = spool.tile([P, 1], f32, tag="den2")
                nc.vector.tensor_scalar_add(out=den[:], in0=acc[:, 64:65],
                                            scalar1=EPS)
                rec = spool.tile([P, 1], f32, tag="rec2")
                nc.vector.reciprocal(out=rec[:], in_=den[:])
                o = wpool.tile([P, HID], f32, tag="o2")
                nc.vector.tensor_scalar_mul(out=o[:], in0=acc[:, 0:HID],
                                            scalar1=rec[:, 0:1])
                nc.vector.tensor_tensor(out=o[:], in0=o[:], in1=b2_sb[:],
                                        op=OP.add)
                bv = spool.tile([P, 1], f32, tag="bv")
                nc.sync.dma_start(out=bv[:], in_=batchv[t])
                oh = wpool.tile([P, G], f32, tag="oh")
                nc.vector.tensor_tensor(
                    out=oh[:], in0=bv[:, 0:1].to_broadcast([P, G]),
                    in1=io64_sb[:], op=OP.is_equal)
                rp = wpool.tile([P, HID + 1], f32, tag="rp")
                nc.vector.tensor_copy(out=rp[:, 0:HID], in_=o[:])
                nc.vector.tensor_copy(out=rp[:, HID:HID + 1], in_=ones_sb[:])
                nc.tensor.matmul(out=pool_ps[:], lhsT=oh[:], rhs=rp[:],
                                 start=(t == 0), stop=(t == NT - 1))

            edge_layer(h2_full, h2_shard, R2, 1, 66, 68, post2)

            # ================= pooling reduce + classifier ================
            pool_sb = spool.tile([G, HID + 1], f32, tag="pool_sb")
            nc.vector.tensor_copy(out=pool_sb[:], in_=pool_ps[:])
            nc.sync.dma_start(out=pool_in[:], in_=pool_sb[:])
            nc.gpsimd.collective_compute(
                "AllReduce", mybir.AluOpType.add,
                replica_groups=groups,
                ins=[pool_in.opt()], outs=[pool_out.opt()])
            pr = spool.tile([G, HID + 1], f32, tag="pr")
            nc.sync.dma_start(out=pr[:], in_=pool_out[:])
            c1 = spool.tile([G, 1], f32, tag="c1")
            nc.vector.tensor_scalar_max(out=c1[:], in0=pr[:, HID:HID + 1],
                                        scalar1=1.0)
            rc = spool.tile([G, 1], f32, tag="rc")
            nc.vector.reciprocal(out=rc[:], in_=c1[:])
            pooled = spool.tile([G, HID], f32, tag="pooled")
            nc.vector.tensor_scalar_mul(out=pooled[:], in0=pr[:, 0:HID],
                                        scalar1=rc[:, 0:1])
            pT = pcls.tile([P, P], f32, tag="cls")
            nc.tensor.transpose(out=pT[0:HID, 0:G], in_=pooled[:],
                                identity=idf_sb[0:G, 0:G])
            pT_sb = spool.tile([HID, G], f32, tag="pT_sb")
            nc.vector.tensor_copy(out=pT_sb[:], in_=pT[0:HID, 0:G])
            lgT = pcls.tile([P, P], f32, tag="cls")
            nc.tensor.matmul(out=lgT[0:NCLS, 0:G], lhsT=wc_sb[:], rhs=pT_sb[:],
                             start=True, stop=True)
            lgT_sb = spool.tile([NCLS, G], f32, tag="lgT_sb")
            nc.vector.tensor_copy(out=lgT_sb[:], in_=lgT[0:NCLS, 0:G])
            lg_ps = pcls.tile([P, P], f32, tag="cls")
            nc.tensor.transpose(out=lg_ps[0:G, 0:NCLS], in_=lgT_sb[:],
                                identity=idf_sb[0:NCLS, 0:NCLS])
            lg = spool.tile([G, NCLS], f32, tag="lg")
            nc.vector.tensor_tensor(out=lg[:], in0=lg_ps[0:G, 0:NCLS],
                                    in1=bc_sb[:], op=OP.add)
            mx = spool.tile([G, 1], f32, tag="mx")
            nc.vector.tensor_reduce(out=mx[:], in_=lg[:],
                                    axis=mybir.AxisListType.X, op=OP.max)
            tm = spool.tile([G, NCLS], f32, tag="tm")
            nc.vector.tensor_scalar(out=tm[:], in0=lg[:],
                                    scalar1=mx[:, 0:1], scalar2=None,
                                    op0=OP.subtract)
            e2 = spool.tile([G, NCLS], f32, tag="e2")
            nc.scalar.activation(out=e2[:], in_=tm[:], func=AF.Exp)
            sm = spool.tile([G, 1], f32, tag="sm")
            nc.vector.tensor_reduce(out=sm[:], in_=e2[:],
                                    axis=mybir.AxisListType.X, op=OP.add)
            ln = spool.tile([G, 1], f32, tag="ln")
            nc.scalar.activation(out=ln[:], in_=sm[:], func=AF.Ln)
            yt = spool.tile([G, NCLS], f32, tag="yt")
            nc.vector.tensor_scalar(out=yt[:], in0=tm[:],
                                    scalar1=ln[:, 0:1], scalar2=None,
                                    op0=OP.subtract)
            nc.sync.dma_start(out=y[:], in_=yt[:])

    nc.finalize()
    return nc


def kernel(**inputs) -> np.ndarray:
    from concourse import bass_utils

    cfg, per_core = host_prep(inputs, cores=8)
    nc = build_program(cfg)
    res = bass_utils.run_bass_kernel_spmd(
        nc, per_core, core_ids=list(range(cfg["cores"])))
    return np.asarray(res.results[0]["y"])


if __name__ == "__main__":
    import reference
    ins = reference.setup_inputs()
    out = kernel(**{k: np.asarray(v) for k, v in ins.items()})
    exp = np.asarray(reference.reference(**ins))
    err = np.abs(out - exp).max() / max(np.abs(exp).max(), 1e-12)
    print("Relative error:", err)


# revision 8
# speedup vs baseline: 1.1712x; 1.1712x over previous
"""Two-layer GAT (PyG GATConv semantics) on 8 Trainium2 NeuronCores.

v3 design:
- Edge/dst sharding: host sorts edges by dst; core c owns dst nodes
  [c*SHARD,(c+1)*SHARD) and their incoming edges, grouped per
  128-dst-node tile into KT subtiles of 128 edge slots.
- Batched gathers via InstDMAGatherAnt (nc.gpsimd.dma_gather): one SWDGE
  instruction per (tile, half-table group) instead of one per subtile.
  dma_gather needs int16 indices, so the node table is split at
  HALF=NP/2 and each tile's edges are regrouped: slots [0,KA) gather
  from rows <HALF, slots [KA,KT) from rows >=HALF (index biased).
  Index layout: k-major flat order wrapped into 16 partitions and
  replicated 8x down partitions (one copy per GpSimd q7 core).
- Table rows are 256B-multiple (gather constraint):
  layer1 [NP,256] bf16: [h0(64)|1|h1(64)|1|as 2xf32(4)|ad 2xbf16(2)|pad]
  layer2 [NP,128] bf16: [h2(64)|1|pad|as2 f32(2)|ad2 bf16(1)|pad]
  The literal 1.0 columns make [h|1] contiguous so one matmul computes
  segment numerator and denominator together.
- ad[dst] expansion: host ships transposed one-hots st2 (bf16 0/1,
  [NT,P,KT*P]); per subtile one tiny PE matmul st2_k.T @ adn. ad is
  stored bf16: softmax is shift-invariant per segment so ad rounding
  only acts through the leaky-relu kink.
- z = as[src]+ad[dst] fp32 via bitcast strided views, one DVE op/tile;
  Lrelu+Exp whole-tile on the scalar engine.
- Scaled one-hot fused in one DVE op per (subtile, head):
  st = (iota == dst_rel) * ex; acc matmul bf16 (1cy/row).
- Phase0 row-major via matmul(lhsT=xT_tile, rhs=W), no transposes.
- AllGathers bf16 into Shared scratchpad.
"""

import sys

for _p in ("/opt/trn_rl_repo",):
    if _p not in sys.path:
        sys.path.insert(0, _p)

import numpy as np
import ml_dtypes

P = 128
NEG_SLOPE = 0.2
EPS = 1e-16
R1 = 256          # layer-1 gather row elems (bf16)
C1 = 136          # layer-1 compact row elems actually written
R2 = 128          # layer-2 gather row elems
C2 = 70           # layer-2 compact row elems (even pitch)
G = 64
NQ = 4            # SWDGE queues


def _to_bf(a):
    return np.asarray(a, np.float32).astype(ml_dtypes.bfloat16)


def _wrap16(mat):
    """[T, n] k-major flat idx -> [T, 128, n/16] (16-wrap, 8x replicated)."""
    T, n = mat.shape
    blk = mat.reshape(T, n // 16, 16).transpose(0, 2, 1).astype(np.int16)
    return np.ascontiguousarray(np.tile(blk, (1, 8, 1)))


def host_prep(inputs, cores=8):
    x = np.asarray(inputs["x"], dtype=np.float32)
    edge_index = np.asarray(inputs["edge_index"])
    batch = np.asarray(inputs["batch"])
    W1 = np.asarray(inputs["W1"], dtype=np.float32)
    a_src1 = np.asarray(inputs["a_src1"], dtype=np.float32)
    a_dst1 = np.asarray(inputs["a_dst1"], dtype=np.float32)
    b1 = np.asarray(inputs["b1"], dtype=np.float32)
    W2 = np.asarray(inputs["W2"], dtype=np.float32)
    a_src2 = np.asarray(inputs["a_src2"], dtype=np.float32)
    a_dst2 = np.asarray(inputs["a_dst2"], dtype=np.float32)
    b2 = np.asarray(inputs["b2"], dtype=np.float32)
    Wc = np.asarray(inputs["Wc"], dtype=np.float32)
    bc = np.asarray(inputs["bc"], dtype=np.float32)

    N, F = x.shape
    H1 = a_src1.shape[0]            # 2
    HID = a_src1.shape[1]           # 64
    D1 = H1 * HID                   # 128
    NCLS = Wc.shape[1]

    NP = ((N + cores * P - 1) // (cores * P)) * (cores * P)
    SHARD = NP // cores
    NT = SHARD // P

    # --- weights ---
    As1 = np.zeros((D1, H1), np.float32)
    Ad1 = np.zeros((D1, H1), np.float32)
    for h in range(H1):
        As1[h * HID:(h + 1) * HID, h] = a_src1[h]
        Ad1[h * HID:(h + 1) * HID, h] = a_dst1[h]
    Wh = _to_bf(W1)
    Wsd = _to_bf(np.concatenate([W1 @ As1, W1 @ Ad1], axis=1))  # [F,4]
    W2aug = _to_bf(np.concatenate([W2, W2 @ a_src2.T, W2 @ a_dst2.T], axis=1))

    # --- edges sorted by dst, split by src half, grouped per dst tile ---
    src = edge_index[0].astype(np.int64)
    dst = edge_index[1].astype(np.int64)
    order = np.argsort(dst, kind="stable")
    ss = src[order]
    ds = dst[order]
    rel = (ds % P).astype(np.float32)
    tile_of = (ds // P).astype(np.int64)
    n_tiles = NP // P
    counts = np.bincount(tile_of, minlength=n_tiles)
    starts = np.cumsum(counts) - counts

    # N-way split of the node table: dma_gather caps num_idxs at 1024
    # and indices are int16, so each group must satisfy rows<=32767 and
    # max edges/tile/group <= 1024.
    NS = 3
    while True:
        S = -(-NP // NS)
        grp = np.minimum(ss // S, NS - 1)
        nQ = np.zeros((n_tiles, NS), np.int64)
        for t in range(n_tiles):
            nQ[t] = np.bincount(grp[starts[t]:starts[t] + counts[t]],
                                minlength=NS)
        Ks = [int(np.ceil(nQ[:, q].max() / P)) for q in range(NS)]
        if S <= 32767 and all(k <= 8 for k in Ks):
            break
        NS += 1
    KT = sum(Ks)
    koff = np.cumsum([0] + Ks)

    idx_all = [np.zeros((n_tiles, Ks[q] * P), np.int16) for q in range(NS)]
    relfull = np.full((n_tiles, P, KT), 255.0, np.float32)
    for t in range(n_tiles):
        e0 = starts[t]
        e1 = e0 + counts[t]
        s_t = ss[e0:e1]
        r_t = rel[e0:e1]
        g_t = grp[e0:e1]
        for q in range(NS):
            m = g_t == q
            sq = s_t[m] - q * S
            rq = r_t[m]
            idx_all[q][t, :len(sq)] = sq.astype(np.int16)
            iq = np.arange(len(sq))
            relfull[t, iq % P, koff[q] + iq // P] = rq

    # transposed one-hots st2[t][d, k*128+e] = (relfull[t,e,k]==d), bf16
    st2_bool = (np.arange(P, dtype=np.float32)[None, :, None, None]
                == relfull.transpose(0, 2, 1)[:, None, :, :])
    st2_all = st2_bool.astype(ml_dtypes.bfloat16).reshape(n_tiles, P, KT * P)
    del st2_bool

    idx_w = [_wrap16(idx_all[q]) for q in range(NS)]

    bpad = np.full(NP, 255.0, np.float32)
    bpad[:N] = batch.astype(np.float32)
    xpad = np.zeros((NP, F), np.float32)
    xpad[:N] = x

    iota128 = np.tile(np.arange(P, dtype=np.float32), (P, 1))
    iota64 = np.tile(np.arange(G, dtype=np.float32), (P, 1))
    ident_bf = np.eye(P, dtype=np.float32).astype(ml_dtypes.bfloat16)
    ident_f = np.eye(P, dtype=np.float32)
    b1rep = np.tile(b1, (P, 1))
    b2rep = np.tile(b2, (P, 1))
    bcrep = np.tile(bc, (G, 1))
    ones_col = np.ones((P, 1), np.float32)

    shared = {
        "Wh": Wh, "Wsd": Wsd, "W2aug": W2aug, "iota128": iota128,
        "iota64": iota64, "ident_bf": ident_bf, "ident_f": ident_f,
        "b1rep": b1rep, "b2rep": b2rep, "Wc": Wc, "bcrep": bcrep,
        "ones_col": ones_col,
    }
    per_core = []
    for c in range(cores):
        lo = c * SHARD
        t0, t1 = c * NT, (c + 1) * NT
        dr_all = relfull[t0:t1].transpose(1, 0, 2).reshape(P, NT * KT)
        pc = {
            **shared,
            "xT": np.ascontiguousarray(_to_bf(xpad[lo:lo + SHARD].T)),
            "dr_all": np.ascontiguousarray(dr_all),
            "st2": np.ascontiguousarray(st2_all[t0:t1]),
            "batchv": np.ascontiguousarray(
                bpad[lo:lo + SHARD].reshape(NT, P, 1)),
        }
        for q in range(NS):
            iqw = idx_w[q][t0:t1].transpose(1, 0, 2).reshape(
                P, NT * Ks[q] * 8)
            pc[f"idx{q}"] = np.ascontiguousarray(iqw)
        per_core.append(pc)

    cfg = dict(N=N, F=F, H1=H1, HID=HID, D1=D1, NCLS=NCLS, G=G, NP=NP,
               SHARD=SHARD, NT=NT, NS=NS, S=S, Ks=Ks, KT=KT,
               cores=cores)
    return cfg, per_core


def build_program(cfg):
    import concourse.bacc as bacc
    import concourse.bass as bass
    import concourse.mybir as mybir
    import concourse.tile as tile

    f32 = mybir.dt.float32
    bf16 = mybir.dt.bfloat16
    i16 = mybir.dt.int16
    AF = mybir.ActivationFunctionType
    OP = mybir.AluOpType

    F, H1, HID, D1 = cfg["F"], cfg["H1"], cfg["HID"], cfg["D1"]
    NCLS = cfg["NCLS"]
    NP, SHARD, NT = cfg["NP"], cfg["SHARD"], cfg["NT"]
    NS, S, Ks, KT = cfg["NS"], cfg["S"], cfg["Ks"], cfg["KT"]
    koff = [0]
    for k in Ks:
        koff.append(koff[-1] + k)
    cores = cfg["cores"]

    nc = bacc.Bacc("TRN2", target_bir_lowering=False, debug=False,
                   num_swdge_queues=NQ)

    xT = nc.dram_tensor("xT", [F, SHARD], bf16, kind="ExternalInput")
    idx_d = [nc.dram_tensor(f"idx{q}", [P, NT * Ks[q] * 8], i16,
                            kind="ExternalInput") for q in range(NS)]
    dr_all_d = nc.dram_tensor("dr_all", [P, NT * KT], f32,
                              kind="ExternalInput")
    st2_d = nc.dram_tensor("st2", [NT, P, KT * P], bf16,
                           kind="ExternalInput")
    batchv = nc.dram_tensor("batchv", [NT, P, 1], f32, kind="ExternalInput")
    Wh_d = nc.dram_tensor("Wh", [F, D1], bf16, kind="ExternalInput")
    Wsd_d = nc.dram_tensor("Wsd", [F, 4], bf16, kind="ExternalInput")
    W2aug = nc.dram_tensor("W2aug", [D1, HID + 2], bf16, kind="ExternalInput")
    iota128 = nc.dram_tensor("iota128", [P, P], f32, kind="ExternalInput")
    iota64 = nc.dram_tensor("iota64", [P, G], f32, kind="ExternalInput")
    ident_bf = nc.dram_tensor("ident_bf", [P, P], bf16, kind="ExternalInput")
    ident_f = nc.dram_tensor("ident_f", [P, P], f32, kind="ExternalInput")
    b1rep = nc.dram_tensor("b1rep", [P, D1], f32, kind="ExternalInput")
    b2rep = nc.dram_tensor("b2rep", [P, HID], f32, kind="ExternalInput")
    Wc = nc.dram_tensor("Wc", [HID, NCLS], f32, kind="ExternalInput")
    bcrep = nc.dram_tensor("bcrep", [G, NCLS], f32, kind="ExternalInput")
    ones_col = nc.dram_tensor("ones_col", [P, 1], f32, kind="ExternalInput")

    y = nc.dram_tensor("y", [G, NCLS], f32, kind="ExternalOutput")

    def view3(t, off, blk, n, w):
        b = t[:]
        return bass.AP(b.tensor, b.offset + off,
                       [[b.ap[0][0], P], [blk, n], [1, w]])

    with tile.TileContext(nc) as tc:
        with (
            tc.tile_pool(name="const", bufs=1) as cpool,
            tc.tile_pool(name="gat", bufs=3) as gpool,
            tc.tile_pool(name="st2p", bufs=3) as st2pool,
            tc.tile_pool(name="work", bufs=3) as wpool,
            tc.tile_pool(name="small", bufs=3) as spool,
            tc.tile_pool(name="pbig", bufs=2, space="PSUM") as pbig,
            tc.tile_pool(name="pacc", bufs=2, space="PSUM") as pacc,
            tc.tile_pool(name="padx", bufs=2, space="PSUM") as padx,
            tc.tile_pool(name="pcls", bufs=1, space="PSUM") as pcls,
            tc.tile_pool(name="ppool", bufs=1, space="PSUM") as ppool,
            tc.tile_pool(name="dram", bufs=1, space="DRAM") as dpool,
        ):
            def cload(ap, shape, dt, tag):
                t = cpool.tile(shape, dt, tag=tag)
                nc.sync.dma_start(out=t[:], in_=ap[:])
                return t

            wh_sb = cload(Wh_d, [F, D1], bf16, "wh")
            wsd_sb = cload(Wsd_d, [F, 4], bf16, "wsd")
            w2_sb = cload(W2aug, [D1, HID + 2], bf16, "w2")
            io128_sb = cload(iota128, [P, P], f32, "io128")
            io128b_sb = cpool.tile([P, P], bf16, tag="io128b")
            nc.vector.tensor_copy(out=io128b_sb[:], in_=io128_sb[:])
            io64_sb = cload(iota64, [P, G], f32, "io64")
            idb_sb = cload(ident_bf, [P, P], bf16, "idb")
            idf_sb = cload(ident_f, [P, P], f32, "idf")
            b1_sb = cload(b1rep, [P, D1], f32, "b1")
            b2_sb = cload(b2rep, [P, HID], f32, "b2")
            wc_sb = cload(Wc, [HID, NCLS], f32, "wc")
            bc_sb = cload(bcrep, [G, NCLS], f32, "bc")
            ones_sb = cload(ones_col, [P, 1], f32, "ones")
            idx_sb = [cload(idx_d[q], [P, NT * Ks[q] * 8], i16, f"i{q}")
                      for q in range(NS)]
            dr_sb = cload(dr_all_d, [P, NT * KT], f32, "drall")

            h1_shard = dpool.tile([SHARD, R1], bf16, tag="h1s")
            h1_full = dpool.tile([NP, R1], bf16, tag="h1f",
                                 addr_space="Shared")
            h2_shard = dpool.tile([SHARD, R2], bf16, tag="h2s")
            h2_full = dpool.tile([NP, R2], bf16, tag="h2f",
                                 addr_space="Shared")
            pool_in = dpool.tile([G, HID + 1], f32, tag="pin")
            pool_out = dpool.tile([G, HID + 1], f32, tag="pout")

            groups = [list(range(cores))]

            # ===== phase 0: layer-1 table rows = x @ [W1 | W1As | W1Ad] ====
            for t in range(NT):
                xt = wpool.tile([F, P], bf16, tag="xt")
                nc.sync.dma_start(out=xt[:], in_=xT[:, t * P:(t + 1) * P])
                ph = pbig.tile([P, 194], f32, tag="big")
                nc.tensor.matmul(out=ph[:, 0:D1], lhsT=xt[:], rhs=wh_sb[:],
                                 start=True, stop=True)
                nc.tensor.matmul(out=ph[:, D1:D1 + 4], lhsT=xt[:],
                                 rhs=wsd_sb[:], start=True, stop=True)
                row = wpool.tile([P, C1], bf16, tag="row1")
                nc.scalar.activation(out=row[:, 0:64], in_=ph[:, 0:64],
                                     func=AF.Copy)
                nc.scalar.activation(out=row[:, 65:129], in_=ph[:, 64:128],
                                     func=AF.Copy)
                nc.vector.memset(row[:, 64:65], 1.0)
                nc.vector.memset(row[:, 129:130], 1.0)
                nc.scalar.activation(out=row[:, 130:134].bitcast(f32),
                                     in_=ph[:, D1:D1 + 2], func=AF.Copy)
                nc.scalar.activation(out=row[:, 134:136],
                                     in_=ph[:, D1 + 2:D1 + 4], func=AF.Copy)
                nc.sync.dma_start(out=h1_shard[t * P:(t + 1) * P, 0:C1],
                                  in_=row[:])

            nc.gpsimd.collective_compute(
                "AllGather", mybir.AluOpType.bypass,
                replica_groups=groups,
                ins=[h1_shard.opt()], outs=[h1_full.opt()])

            # =================== edge phase (both layers) ==================
            def edge_layer(table_full, shard, R, heads, as_off, ad_off, post):
                tabs = [table_full[q * S:min((q + 1) * S, NP), :]
                        for q in range(NS)]
                for t in range(NT):
                    g = gpool.tile([P, KT * R], bf16, tag=f"g{R}")
                    for q in range(NS):
                        nc.gpsimd.dma_gather(
                            out_ap=view3(g, koff[q] * R, R, Ks[q], R),
                            in_ap=tabs[q],
                            idxs_ap=idx_sb[q][:, t * Ks[q] * 8:
                                              (t + 1) * Ks[q] * 8],
                            num_idxs=Ks[q] * P, num_idxs_reg=Ks[q] * P,
                            elem_size=R, queue_num=(NS * t + q) % NQ,
                            single_packet=False)
                    st2_sb = st2pool.tile([P, KT * P], bf16, tag="st2")
                    nc.sync.dma_start(out=st2_sb[:], in_=st2_d[t])
                    adn = spool.tile([P, heads], bf16, tag=f"adn{R}")
                    nc.sync.dma_start(
                        out=adn[:],
                        in_=shard[t * P:(t + 1) * P, ad_off:ad_off + heads])
                    adx = padx.tile([P, KT * H1], f32, tag="adx")
                    for k in range(KT):
                        nc.tensor.matmul(
                            out=adx[:, k * heads:(k + 1) * heads],
                            lhsT=st2_sb[:, k * P:(k + 1) * P], rhs=adn[:],
                            start=True, stop=True)
                    z = spool.tile([P, KT * heads], f32, tag=f"z{R}")
                    nc.vector.tensor_tensor(
                        out=view3(z, 0, heads, KT, heads),
                        in0=view3(g, as_off, R, KT, 2 * heads).bitcast(f32),
                        in1=view3(adx, 0, heads, KT, heads),
                        op=OP.add)
                    zl = spool.tile([P, KT * heads], f32, tag=f"zl{R}")
                    nc.vector.tensor_scalar_mul(out=zl[:], in0=z[:],
                                                scalar1=NEG_SLOPE)
                    zm = spool.tile([P, KT * heads], f32, tag=f"zm{R}")
                    nc.vector.tensor_tensor(out=zm[:], in0=z[:], in1=zl[:],
                                            op=OP.max)
                    ex = spool.tile([P, KT * heads], f32, tag=f"ex{R}")
                    nc.scalar.activation(out=ex[:], in_=zm[:], func=AF.Exp)
                    acc = pacc.tile([P, H1 * 65], f32, tag="acc")
                    for k in range(KT):
                        if heads == 1:
                            sth = wpool.tile([P, P], bf16, tag="sth")
                            nc.vector.tensor_scalar(
                                out=sth[:], in0=io128b_sb[:],
                                scalar1=dr_sb[:, t * KT + k:t * KT + k + 1],
                                scalar2=ex[:, k:k + 1],
                                op0=OP.is_equal, op1=OP.mult)
                            nc.tensor.matmul(
                                out=acc[:, 0:65], lhsT=sth[:],
                                rhs=g[:, k * R:k * R + 65],
                                start=(k == 0), stop=(k == KT - 1))
                        else:
                            # plain one-hot; fold ex into the rhs [h|1] blocks
                            sth = wpool.tile([P, P], bf16, tag="sth")
                            nc.vector.tensor_scalar(
                                out=sth[:], in0=io128b_sb[:],
                                scalar1=dr_sb[:, t * KT + k:t * KT + k + 1],
                                scalar2=None, op0=OP.is_equal)
                            for h in range(heads):
                                nc.vector.tensor_scalar_mul(
                                    out=g[:, k * R + h * 65:k * R + h * 65 + 65],
                                    in0=g[:, k * R + h * 65:k * R + h * 65 + 65],
                                    scalar1=ex[:, k * heads + h:
                                               k * heads + h + 1])
                            nc.tensor.matmul(
                                out=acc[:], lhsT=sth[:],
                                rhs=g[:, k * R:k * R + 130],
                                start=(k == 0), stop=(k == KT - 1))
                    post(t, acc)

            # ---- layer-1 post: divide, +b1, ELU, project to layer-2 row ---
            def post1(t, acc):
                den = spool.tile([P, H1], f32, tag="den")
                nc.vector.tensor_scalar_add(
                    out=den[:], in0=view3(acc, 64, 65, H1, 1), scalar1=EPS)
                rec = spool.tile([P, H1], f32, tag="rec")
                nc.vector.reciprocal(out=rec[:], in_=den[:])
                o = wpool.tile([P, D1], f32, tag="o")
                for h in range(H1):
                    nc.vector.tensor_scalar_mul(
                        out=o[:, h * 64:(h + 1) * 64],
                        in0=acc[:, h * 65:h * 65 + 64],
                        scalar1=rec[:, h:h + 1])
                nc.vector.tensor_tensor(out=o[:], in0=o[:], in1=b1_sb[:],
                                        op=OP.add)
                m0 = wpool.tile([P, D1], f32, tag="m0")
                nc.vector.tensor_scalar_min(out=m0[:], in0=o[:], scalar1=0.0)
                em = wpool.tile([P, D1], f32, tag="em")
                nc.scalar.activation(out=em[:], in_=m0[:], func=AF.Exp)
                nc.vector.tensor_scalar_add(out=em[:], in0=em[:], scalar1=-1.0)
                hb = wpool.tile([P, D1], f32, tag="hb")
                nc.vector.tensor_tensor(out=hb[:], in0=o[:], in1=em[:],
                                        op=OP.max)
                hT = pbig.tile([P, 194], f32, tag="big")
                nc.tensor.transpose(out=hT[:, 0:P], in_=hb[:],
                                    identity=idf_sb[:])
                hTs = wpool.tile([P, P], bf16, tag="hTs")
                nc.scalar.activation(out=hTs[:], in_=hT[:, 0:P], func=AF.Copy)
                nc.tensor.matmul(out=hT[:, 128:128 + HID + 2], lhsT=hTs[:],
                                 rhs=w2_sb[:], start=True, stop=True)
                row2 = wpool.tile([P, C2], bf16, tag="row2")
                nc.scalar.activation(out=row2[:, 0:64],
                                     in_=hT[:, 128:128 + 64], func=AF.Copy)
                nc.vector.memset(row2[:, 64:65], 1.0)
                nc.vector.memset(row2[:, 65:66], 0.0)
                nc.vector.memset(row2[:, 69:70], 0.0)
                nc.scalar.activation(
                    out=row2[:, 66:68].bitcast(f32),
                    in_=hT[:, 128 + HID:128 + HID + 1], func=AF.Copy)
                nc.scalar.activation(
                    out=row2[:, 68:69],
                    in_=hT[:, 128 + HID + 1:128 + HID + 2], func=AF.Copy)
                nc.sync.dma_start(out=h2_shard[t * P:(t + 1) * P, 0:C2],
                                  in_=row2[:])

            edge_layer(h1_full, h1_shard, R1, H1, 130, 134, post1)

            nc.gpsimd.collective_compute(
                "AllGather", mybir.AluOpType.bypass,
                replica_groups=groups,
                ins=[h2_shard.opt()], outs=[h2_full.opt()])

            # ---- layer-2 post: divide, +b2, pool accumulate ----
            pool_ps = ppool.tile([G, HID + 1], f32, tag="pool_ps")

            def post2(t, acc):
                den = spool.tile([P, 1], f32, tag="den2")
                nc.vector.tensor_scalar_add(out=den[:], in0=acc[:, 64:65],
                                            scalar1=EPS)
                rec = spool.tile([P, 1], f32, tag="rec2")
                nc.vector.reciprocal(out=rec[:], in_=den[:])
                o = wpool.tile([P, HID], f32, tag="o2")
                nc.vector.tensor_scalar_mul(out=o[:], in0=acc[:, 0:HID],
                                            scalar1=rec[:, 0:1])
                nc.vector.tensor_tensor(out=o[:], in0=o[:], in1=b2_sb[:],
                                        op=OP.add)
                bv = spool.tile([P, 1], f32, tag="bv")
                nc.sync.dma_start(out=bv[:], in_=batchv[t])
                oh = wpool.tile([P, G], f32, tag="oh")
                nc.vector.tensor_tensor(
                    out=oh[:], in0=bv[:, 0:1].to_broadcast([P, G]),
                    in1=io64_sb[:], op=OP.is_equal)
                rp = wpool.tile([P, HID + 1], f32, tag="rp")
                nc.scalar.activation(out=rp[:, 0:HID], in_=o[:], func=AF.Copy)
                nc.scalar.activation(out=rp[:, HID:HID + 1], in_=ones_sb[:],
                                     func=AF.Copy)
                nc.tensor.matmul(out=pool_ps[:], lhsT=oh[:], rhs=rp[:],
                                 start=(t == 0), stop=(t == NT - 1))

            edge_layer(h2_full, h2_shard, R2, 1, 66, 68, post2)

            # ================= pooling reduce + classifier ================
            pool_sb = spool.tile([G, HID + 1], f32, tag="pool_sb")
            nc.vector.tensor_copy(out=pool_sb[:], in_=pool_ps[:])
            nc.sync.dma_start(out=pool_in[:], in_=pool_sb[:])
            nc.gpsimd.collective_compute(
                "AllReduce", mybir.AluOpType.add,
                replica_groups=groups,
                ins=[pool_in.opt()], outs=[pool_out.opt()])
            pr = spool.tile([G, HID + 1], f32, tag="pr")
            nc.sync.dma_start(out=pr[:], in_=pool_out[:])
            c1 = spool.tile([G, 1], f32, tag="c1")
            nc.vector.tensor_scalar_max(out=c1[:], in0=pr[:, HID:HID + 1],
                                        scalar1=1.0)
            rc = spool.tile([G, 1], f32, tag="rc")
            nc.vector.reciprocal(out=rc[:], in_=c1[:])
            pooled = spool.tile([G, HID], f32, tag="pooled")
            nc.vector.tensor_scalar_mul(out=pooled[:], in0=pr[:, 0:HID],
                                        scalar1=rc[:, 0:1])
            pT = pcls.tile([P, P], f32, tag="cls")
            nc.tensor.transpose(out=pT[0:HID, 0:G], in_=pooled[:],
                                identity=idf_sb[0:G, 0:G])
            pT_sb = spool.tile([HID, G], f32, tag="pT_sb")
            nc.vector.tensor_copy(out=pT_sb[:], in_=pT[0:HID, 0:G])
            lgT = pcls.tile([P, P], f32, tag="cls")
            nc.tensor.matmul(out=lgT[0:NCLS, 0:G], lhsT=wc_sb[:], rhs=pT_sb[:],
                             start=True, stop=True)
            lgT_sb = spool.tile([NCLS, G], f32, tag="lgT_sb")
            nc.vector.tensor_copy(out=lgT_sb[:], in_=lgT[0:NCLS, 0:G])
            lg_ps = pcls.tile([P, P], f32, tag="cls")
            nc.tensor.transpose(out=lg_ps[0:G, 0:NCLS], in_=lgT_sb[:],
                                identity=idf_sb[0:NCLS, 0:NCLS])
            lg = spool.tile([G, NCLS], f32, tag="lg")
            nc.vector.tensor_tensor(out=lg[:], in0=lg_ps[0:G, 0:NCLS],
                                    in1=bc_sb[:], op=OP.add)
            mx = spool.tile([G, 1], f32, tag="mx")
            nc.vector.tensor_reduce(out=mx[:], in_=lg[:],
                                    axis=mybir.AxisListType.X, op=OP.max)
            tm = spool.tile([G, NCLS], f32, tag="tm")
            nc.vector.tensor_scalar(out=tm[:], in0=lg[:],
                                    scalar1=mx[:, 0:1], scalar2=None,
                                    op0=OP.subtract)
            e2 = spool.tile([G, NCLS], f32, tag="e2")
            nc.scalar.activation(out=e2[:], in_=tm[:], func=AF.Exp)
            sm = spool.tile([G, 1], f32, tag="sm")
            nc.vector.tensor_reduce(out=sm[:], in_=e2[:],
                                    axis=mybir.AxisListType.X, op=OP.add)
            ln = spool.tile([G, 1], f32, tag="ln")
            nc.scalar.activation(out=ln[:], in_=sm[:], func=AF.Ln)
            yt = spool.tile([G, NCLS], f32, tag="yt")
            nc.vector.tensor_scalar(out=yt[:], in0=tm[:],
                                    scalar1=ln[:, 0:1], scalar2=None,
                                    op0=OP.subtract)
            nc.sync.dma_start(out=y[:], in_=yt[:])

    nc.finalize()
    return nc


def kernel(**inputs) -> np.ndarray:
    from concourse import bass_utils

    cfg, per_core = host_prep(inputs, cores=8)
    nc = build_program(cfg)
    res = bass_utils.run_bass_kernel_spmd(
        nc, per_core, core_ids=list(range(cfg["cores"])))
    return np.asarray(res.results[0]["y"])


if __name__ == "__main__":
    import reference
    ins = reference.setup_inputs()
    out = kernel(**{k: np.asarray(v) for k, v in ins.items()})
    exp = np.asarray(reference.reference(**ins))
    err = np.abs(out - exp).max() / max(np.abs(exp).max(), 1e-12)
    print("Relative error:", err)
